# revision 7
# baseline (speedup 1.0000x reference)
"""Causal dilated conv1d (K=3, dilation=2, N=128 channels) on Trainium2.

out[b,t,i] = sum_{j,k} x[b, t-2k, j] * weight[i,j,k] + bias[i]

Strategy (8-core SPMD, pure data parallel over batch):
  - each core handles 4 of the 32 batch rows; weight/bias replicated
  - on-chip, per batch row: PE-transpose x into a [128(j), T+4] "strip"
    (4-col zero halo on the left so the dilated taps become plain column
    offsets), then 3 accumulated float32r matmuls with 512-wide moving
    operand compute out_T[i, t] = sum_k w_k^T @ xT[:, t-2k], ACT adds the
    per-partition bias while copying PSUM->SBUF, and PE transposes the
    result back to [t, i] layout for contiguous DMA out.
"""

import os
import threading

import numpy as np

import concourse.bass as bass  # noqa: F401  (bass types used via bacc/tile)
import concourse.mybir as mybir
import concourse.tile as tile
from concourse import bacc
from concourse.bass_utils import run_bass_kernel_spmd
from concourse.masks import make_identity

P = 128
KTAPS = 3
DIL = 2
HALO = (KTAPS - 1) * DIL  # 4
NCORES = 8
B_FULL, T_FULL = 32, 8192
B_CORE = B_FULL // NCORES  # 4

FP32 = mybir.dt.float32


def build(Bc=B_CORE, T=T_FULL, chunk=2048, tap_dtype=mybir.dt.float32r):
    """Build the per-core Bass module. Same NEFF runs SPMD on all 8 cores."""
    nc = bacc.Bacc(
        "TRN2",
        target_bir_lowering=False,
        debug=False,
        enable_asserts=False,
        num_devices=NCORES,
    )
    x_d = nc.dram_tensor("x", [Bc, T, P], tap_dtype, kind="ExternalInput")
    w_d = nc.dram_tensor("w", [P, KTAPS * P], tap_dtype, kind="ExternalInput")
    b_d = nc.dram_tensor("b", [P, 1], FP32, kind="ExternalInput")
    o_d = nc.dram_tensor("o", [Bc, T, P], FP32, kind="ExternalOutput")

    x_ap, o_ap = x_d.ap(), o_d.ap()
    n_chunks = T // chunk
    SW = 512  # tap-matmul moving width (1 PSUM bank of fp32)
    S = chunk // SW  # strips per chunk
    CPS = SW // P  # 128-subtiles per strip

    with tile.TileContext(nc) as tc:
        with (
            tc.tile_pool(name="const", bufs=1) as cp,
            tc.tile_pool(name="xn", bufs=3) as xp,
            tc.tile_pool(name="strip", bufs=2) as sp,
            tc.tile_pool(name="oT", bufs=3) as otp,
            tc.tile_pool(name="oc", bufs=3) as ocp,
            tc.tile_pool(name="pxt", bufs=3, space="PSUM") as pxtp,
            tc.tile_pool(name="pacc", bufs=2, space="PSUM") as paccp,
            tc.tile_pool(name="pto", bufs=2, space="PSUM") as ptop,
        ):
            ident = cp.tile([P, P], FP32)
            make_identity(nc, ident)
            # f32r copy of the identity for the (faster) f32r transpose-in path;
            # produced via DVE copy since memset/affine_select can't emit f32r.
            ident_r = cp.tile([P, P], tap_dtype)
            nc.vector.tensor_copy(ident_r[:], ident[:])
            w_sb = cp.tile([P, KTAPS * P], tap_dtype)
            nc.sync.dma_start(w_sb[:], w_d.ap())
            bias_sb = cp.tile([P, 1], FP32)
            nc.sync.dma_start(bias_sb[:], b_d.ap())
            zhalo = cp.tile([P, HALO], FP32)
            nc.vector.memset(zhalo[:], 0.0)

            for b in range(Bc):
                strip = sp.tile([P, T + HALO], tap_dtype, tag="strip")
                nc.vector.tensor_copy(strip[:, 0:HALO], zhalo[:])
                R = chunk // P  # rows of x per partition in the contiguous load
                for ci in range(n_chunks):
                    t0 = ci * chunk
                    # contiguous load: partition p holds x rows t0+p*R .. t0+p*R+R-1
                    xn = xp.tile([P, chunk], tap_dtype, tag="xn")
                    nc.sync.dma_start(
                        xn[:],
                        x_ap[b, t0 : t0 + chunk, :].rearrange("(p f) j -> p (f j)", p=P),
                    )
                    # strip view with col index split as p*R + r
                    sv = strip[:, HALO + t0 : HALO + t0 + chunk].rearrange(
                        "n (p r) -> n r p", p=P
                    )
                    for q in range(chunk // SW):
                        # transpose rows r=4q..4q+3 -> psum; each transpose yields
                        # xT[j, t] for t = t0 + p*R + r (p on the free axis)
                        pxt = pxtp.tile([P, SW], tap_dtype, tag="pxt")
                        for c in range(SW // P):
                            r = q * (SW // P) + c
                            nc.tensor.transpose(
                                pxt[:, c * P : (c + 1) * P],
                                xn[:, r * P : (r + 1) * P],
                                ident_r,
                            )
                        nc.vector.tensor_copy(
                            sv[:, q * (SW // P) : (q + 1) * (SW // P), :],
                            pxt.rearrange("n (c p) -> n c p", c=SW // P),
                        )
                    oc = ocp.tile([P, chunk], FP32, tag="oc")
                    for s in range(S):
                        st = t0 + s * SW
                        # --- 3 dilated taps, accumulated in PSUM ---
                        pacc = paccp.tile([P, SW], FP32, tag="pacc")
                        for k in range(KTAPS):
                            off = HALO + st - DIL * k
                            nc.tensor.matmul(
                                pacc[:],
                                w_sb[:, k * P : (k + 1) * P],
                                strip[:, off : off + SW],
                                start=(k == 0),
                                stop=(k == KTAPS - 1),
                            )
                        # --- bias during PSUM->SBUF copy (bias is per-partition here) ---
                        oT = otp.tile([P, SW], FP32, tag="oT")
                        nc.scalar.add(oT[:], pacc[:], bias_sb[:])
                        # --- transpose back to [t, i] ---
                        pto = ptop.tile([P, SW], FP32, tag="pto")
                        for c in range(CPS):
                            nc.tensor.transpose(
                                pto[:, c * P : (c + 1) * P],
                                oT[:, c * P : (c + 1) * P],
                                ident,
                            )
                        nc.vector.tensor_copy(oc[:, s * SW : (s + 1) * SW], pto[:])
                    nc.sync.dma_start(
                        o_ap[b, t0 : t0 + chunk, :].rearrange("(c p) j -> p c j", p=P),
                        oc.rearrange("p (c j) -> p c j", j=P),
                    )
    nc.compile()
    return nc


_cache = {}
_lock = threading.Lock()


def _get_nc():
    with _lock:
        if "nc" not in _cache:
            tap = os.environ.get("CONV_TAP_DTYPE", "float32r")
            _cache["nc"] = build(tap_dtype=getattr(mybir.dt, tap))
        return _cache["nc"]


def prep_inputs(x, weight, bias):
    # w_all[j, k*128 + i] = weight[i, j, k]
    w_all = np.ascontiguousarray(
        np.transpose(np.asarray(weight, np.float32), (1, 2, 0)).reshape(P, KTAPS * P)
    )
    b2 = np.ascontiguousarray(np.asarray(bias, np.float32).reshape(P, 1))
    return np.ascontiguousarray(np.asarray(x, np.float32)), w_all, b2


def kernel(x, weight, bias, _trace=False):
    x, w_all, b2 = prep_inputs(x, weight, bias)
    nc = _get_nc()
    in_maps = [
        {"x": x[c * B_CORE : (c + 1) * B_CORE], "w": w_all, "b": b2}
        for c in range(NCORES)
    ]
    res = run_bass_kernel_spmd(nc, in_maps, core_ids=list(range(NCORES)), trace=_trace)
    out = np.concatenate([r["o"] for r in res.results], axis=0)
    if _trace:
        kernel.last_results = res
    return out


# revision 8
# speedup vs baseline: 1.0838x; 1.0838x over previous
"""Causal dilated conv1d (K=3, dilation=2, N=128 channels) on Trainium2.

out[b,t,i] = sum_{j,k} x[b, t-2k, j] * weight[i,j,k] + bias[i]

Strategy (8-core SPMD, pure data parallel over batch):
  - each core handles 4 of the 32 batch rows; weight/bias replicated
  - on-chip, per batch row: PE-transpose x into a [128(j), T+4] "strip"
    (4-col zero halo on the left so the dilated taps become plain column
    offsets), then 3 accumulated float32r matmuls with 512-wide moving
    operand compute out_T[i, t] = sum_k w_k^T @ xT[:, t-2k], ACT adds the
    per-partition bias while copying PSUM->SBUF, and PE transposes the
    result back to [t, i] layout for contiguous DMA out.
"""

import os
import threading

import numpy as np

import concourse.bass as bass  # noqa: F401  (bass types used via bacc/tile)
import concourse.mybir as mybir
import concourse.tile as tile
from concourse import bacc
from concourse.bass_utils import run_bass_kernel_spmd
from concourse.masks import make_identity

P = 128
KTAPS = 3
DIL = 2
HALO = (KTAPS - 1) * DIL  # 4
NCORES = 8
B_FULL, T_FULL = 32, 8192
B_CORE = B_FULL // NCORES  # 4

FP32 = mybir.dt.float32


def build(Bc=B_CORE, T=T_FULL, chunk=2048, tap_dtype=mybir.dt.float32r):
    """Build the per-core Bass module. Same NEFF runs SPMD on all 8 cores."""
    nc = bacc.Bacc(
        "TRN2",
        target_bir_lowering=False,
        debug=False,
        enable_asserts=False,
        num_devices=NCORES,
    )
    x_d = nc.dram_tensor("x", [Bc, T, P], tap_dtype, kind="ExternalInput")
    w_d = nc.dram_tensor("w", [P, KTAPS * P], tap_dtype, kind="ExternalInput")
    b_d = nc.dram_tensor("b", [P, 1], FP32, kind="ExternalInput")
    o_d = nc.dram_tensor("o", [Bc, T, P], FP32, kind="ExternalOutput")

    x_ap, o_ap = x_d.ap(), o_d.ap()
    n_chunks = T // chunk
    SW = 512  # tap-matmul moving width (1 PSUM bank of fp32)
    S = chunk // SW  # strips per chunk
    CPS = SW // P  # 128-subtiles per strip

    with tile.TileContext(nc) as tc:
        with (
            tc.tile_pool(name="const", bufs=1) as cp,
            tc.tile_pool(name="xn", bufs=3) as xp,
            tc.tile_pool(name="strip", bufs=2) as sp,
            tc.tile_pool(name="oT", bufs=3) as otp,
            tc.tile_pool(name="oc", bufs=3) as ocp,
            tc.tile_pool(name="pxt", bufs=3, space="PSUM") as pxtp,
            tc.tile_pool(name="pacc", bufs=2, space="PSUM") as paccp,
            tc.tile_pool(name="pto", bufs=2, space="PSUM") as ptop,
        ):
            ident = cp.tile([P, P], FP32)
            make_identity(nc, ident)
            # f32r copy of the identity for the (faster) f32r transpose-in path;
            # produced via DVE copy since memset/affine_select can't emit f32r.
            ident_r = cp.tile([P, P], tap_dtype)
            nc.vector.tensor_copy(ident_r[:], ident[:])
            w_sb = cp.tile([P, KTAPS * P], tap_dtype)
            nc.sync.dma_start(w_sb[:], w_d.ap())
            bias_sb = cp.tile([P, 1], FP32)
            nc.sync.dma_start(bias_sb[:], b_d.ap())
            zhalo = cp.tile([P, HALO], FP32)
            nc.vector.memset(zhalo[:], 0.0)

            for b in range(Bc):
                strip = sp.tile([P, T + HALO], tap_dtype, tag="strip")
                nc.vector.tensor_copy(strip[:, 0:HALO], zhalo[:])
                R = chunk // P  # out rows per partition in the contiguous store
                for ci in range(n_chunks):
                    t0 = ci * chunk
                    # load so partition p holds x rows {t0+c*128+p}: consecutive-t
                    # 128-blocks feed the transposes directly
                    xn = xp.tile([P, chunk], tap_dtype, tag="xn")
                    nc.sync.dma_start(
                        xn.rearrange("p (c j) -> p c j", j=P),
                        x_ap[b, t0 : t0 + chunk, :].rearrange("(c p) j -> p c j", p=P),
                    )
                    # out_T accumulator for the whole chunk: [i, t-t0]
                    oT = otp.tile([P, chunk], FP32, tag="oT")
                    for s in range(S):
                        st = t0 + s * SW
                        # --- transpose x subtiles into the strip ---
                        pxt = pxtp.tile([P, SW], tap_dtype, tag="pxt")
                        for c in range(CPS):
                            cc = s * CPS + c
                            nc.tensor.transpose(
                                pxt[:, c * P : (c + 1) * P],
                                xn[:, cc * P : (cc + 1) * P],
                                ident_r,
                            )
                        nc.vector.tensor_copy(
                            strip[:, HALO + st : HALO + st + SW], pxt[:]
                        )
                        # --- 3 dilated taps, accumulated in PSUM ---
                        pacc = paccp.tile([P, SW], FP32, tag="pacc")
                        for k in range(KTAPS):
                            off = HALO + st - DIL * k
                            nc.tensor.matmul(
                                pacc[:],
                                w_sb[:, k * P : (k + 1) * P],
                                strip[:, off : off + SW],
                                start=(k == 0),
                                stop=(k == KTAPS - 1),
                            )
                        # --- bias during PSUM->SBUF copy (bias is per-partition here) ---
                        nc.scalar.add(oT[:, s * SW : (s + 1) * SW], pacc[:], bias_sb[:])
                    # --- transpose back to [t, i], permuted so that partition p of
                    # oc holds out rows t0+p*R .. t0+p*R+R-1 (contiguous 8KB store) ---
                    oTv = oT.rearrange("n (p r) -> n r p", p=P)  # col = p*R + r
                    oc = ocp.tile([P, chunk], FP32, tag="oc")
                    for g in range(chunk // SW):
                        pto = ptop.tile([P, SW], FP32, tag="pto")
                        for c in range(CPS):
                            r = g * CPS + c
                            nc.tensor.transpose(
                                pto[:, c * P : (c + 1) * P], oTv[:, r, :], ident
                            )
                        nc.vector.tensor_copy(
                            oc[:, g * SW : (g + 1) * SW], pto[:]
                        )
                    nc.sync.dma_start(
                        o_ap[b, t0 : t0 + chunk, :].rearrange("(p f) j -> p (f j)", p=P),
                        oc[:],
                    )
    nc.compile()
    return nc


_cache = {}
_lock = threading.Lock()


def _get_nc():
    with _lock:
        if "nc" not in _cache:
            tap = os.environ.get("CONV_TAP_DTYPE", "float32r")
            _cache["nc"] = build(tap_dtype=getattr(mybir.dt, tap))
        return _cache["nc"]


def prep_inputs(x, weight, bias):
    # w_all[j, k*128 + i] = weight[i, j, k]
    w_all = np.ascontiguousarray(
        np.transpose(np.asarray(weight, np.float32), (1, 2, 0)).reshape(P, KTAPS * P)
    )
    b2 = np.ascontiguousarray(np.asarray(bias, np.float32).reshape(P, 1))
    return np.ascontiguousarray(np.asarray(x, np.float32)), w_all, b2


def kernel(x, weight, bias, _trace=False):
    x, w_all, b2 = prep_inputs(x, weight, bias)
    nc = _get_nc()
    in_maps = [
        {"x": x[c * B_CORE : (c + 1) * B_CORE], "w": w_all, "b": b2}
        for c in range(NCORES)
    ]
    res = run_bass_kernel_spmd(nc, in_maps, core_ids=list(range(NCORES)), trace=_trace)
    out = np.concatenate([r["o"] for r in res.results], axis=0)
    if _trace:
        kernel.last_results = res
    return out
